# revision 2
# baseline (speedup 1.0000x reference)
"""Trainium2 Bass kernel for nn_CustomLoss_50843822850472.

Computes, for L2-normalized rows f of `features` [8192, 128]:
    sim = f @ f.T                      (diagonal excluded)
    e   = exp((sim - c) / TAU)         (c = shift center, host rescales)
    S_i = sum_j e_ij                   (total mass, diag excluded)
    M_i = max_j e_ij                   (positive-row detector)
    loss = mean_i [ log(den_i) - log(num_i) ]

Rows are split across 8 NeuronCores (1024 rows/core, 8 blocks of 128).
Each core gets the full feature matrix pre-transposed to [D=128, N=8192]
fp16 and column-rotated by its row offset (SPMD: diag block of row-block m
lands at local columns [m*128, m*128+128), always inside chunk 0).

Per row-block m (PSUM chunks of 2048 cols, double-buffered):
  - chunks 0,1,3 -> ACT: E = exp((sim-c)/TAU) fp16 with fused row-sum accum
    (chunk 0 carries an accumulate-matmul adding -60000*I on the diagonal
    128 cols, so exp -> 0 and the diagonal drops out of all reductions)
  - chunk 2 -> DVE Schraudolph: bits = u16(sim*slope + B0) (saturating
    convert: negatives clamp to 0 = fp16 zero; rounding is RNE), giving
    E' ~ exp((sim-c)/TAU) as the fp16 view of the bits; then a 4x-mode
    tensor_scalar sum pass accumulates its row sums.
  - one 4x-mode tensor_scalar pass over the whole row with op1=max
    accumulates M = rowwise max(E) (DVE accumulator follows op1).

Host: S = (sum of accums)*exp(c/TAU); rows with M >= margin*exp((alpha-c)/TAU)
might contain positives (sim >= alpha) and are recomputed exactly in fp64
(rare: ~1% of rows for alpha=0.5); all other rows have P = 0 exactly.
"""
import sys

sys.path.insert(0, "/opt/trn_rl_repo")

import numpy as np

TAU = 0.07
EPS = 1e-10
DIAG_NEG = -60000.0

N = 8192
D = 128
NCORES = 8
R = N // NCORES          # rows per core
NBLK = R // 128          # row blocks per core
CHUNK = 2048             # columns per PSUM chunk (4 banks)
NCHUNK = N // CHUNK
DVE_CHUNK = 2            # chunk index handled by the DVE exp path
LOG2E = float(np.log2(np.e))
_CACHE = {}
LAST_RESULT = None
PROFILE = False

# accumulator column layout: [128, NACC]
#   cols [0, 3*NBLK)          ACT row-sum partials, 3 per block
#   cols [3*NBLK, 4*NBLK)     DVE-chunk row-sum partials, 1 per block
#   cols [4*NBLK, 5*NBLK)     row-max partials, 1 per block
NACC = 5 * NBLK


def _shift_center(alpha: float) -> float:
    # E = exp((sim - c)/TAU) must fit fp16 (and the Schraudolph bits must
    # stay below fp16-inf = 31744): sim <= ~1.0002 needs c >= ~0.23.
    return float(min(max(alpha, 0.30), 1.0))


def _build(alpha: float):
    import concourse.mybir as mybir
    from concourse import bacc, tile

    f32 = mybir.dt.float32
    f16 = mybir.dt.float16
    u16 = mybir.dt.uint16
    Alu = mybir.AluOpType

    c = _shift_center(alpha)
    bias = float(-c / TAU)
    slope = float(1024.0 * LOG2E / TAU)
    b0 = float(15360.0 - c * slope - 44.5)

    nc = bacc.Bacc(
        "TRN2", target_bir_lowering=False, debug=False, num_devices=NCORES
    )
    ft_d = nc.dram_tensor("ft", [128, N], f16, kind="ExternalInput")
    ident_d = nc.dram_tensor("ident", [128, 128], f16, kind="ExternalInput")
    negd_d = nc.dram_tensor("negd", [128, 128], f16, kind="ExternalInput")
    out_d = nc.dram_tensor("outPS", [128, NACC], f32, kind="ExternalOutput")

    with tile.TileContext(nc) as tc:
        with (
            tc.tile_pool(name="sb", bufs=1) as sb,
            tc.tile_pool(name="ep", bufs=2) as ep,
            tc.tile_pool(name="pp", bufs=2, space="PSUM") as pp,
        ):
            # ft piece 0 first so the first matmuls can start early.
            ft = sb.tile([128, N], f16)
            pieces = [(0, 512), (512, 2048), (2048, 4096), (4096, 6144),
                      (6144, 8192)]
            nc.sync.dma_start(ft[:, 0:512], ft_d[:, 0:512])
            ident = sb.tile([128, 128], f16)
            nc.scalar.dma_start(ident[:], ident_d[:])
            negd = sb.tile([128, 128], f16)
            nc.scalar.dma_start(negd[:], negd_d[:])
            for lo, hi in pieces[1:]:
                nc.sync.dma_start(ft[:, lo:hi], ft_d[:, lo:hi])

            biast = sb.tile([128, 1], f32)
            nc.vector.memset(biast[:], bias)

            acc = sb.tile([128, NACC], f32)

            for m in range(NBLK):
                E = ep.tile([128, N], f16)
                d0 = m * 128
                qd = d0 // 512
                nact = 0
                for k in range(NCHUNK):
                    ps = pp.tile([128, CHUNK], f32, tag="ps")
                    for q in range(CHUNK // 512):
                        nc.tensor.matmul(
                            ps[:, q * 512:(q + 1) * 512],
                            lhsT=ft[:, m * 128:(m + 1) * 128],
                            rhs=ft[:, k * CHUNK + q * 512:k * CHUNK + (q + 1) * 512],
                            start=True,
                            stop=not (k == 0 and q == qd),
                        )
                        if k == 0 and q == qd:
                            # accumulate -60000 onto the diagonal 128 cols
                            nc.tensor.matmul(
                                ps[:, d0:d0 + 128],
                                lhsT=ident[:], rhs=negd[:],
                                start=False, stop=True,
                            )
                    col = k * CHUNK
                    if k == DVE_CHUNK:
                        # Schraudolph exp: u16(sim*slope + b0), saturating
                        nc.vector.tensor_scalar(
                            out=E[:, col:col + CHUNK].bitcast(u16),
                            in0=ps[:],
                            scalar1=slope, scalar2=b0,
                            op0=Alu.mult, op1=Alu.add,
                        )
                        # row-sum of the fp16 view (4x mode, accum=add)
                        nc.vector.tensor_scalar(
                            out=E[:, col:col + CHUNK],
                            in0=E[:, col:col + CHUNK],
                            scalar1=1.0, scalar2=0.0,
                            op0=Alu.mult, op1=Alu.add,
                            accum_out=acc[:, 3 * NBLK + m:3 * NBLK + m + 1],
                        )
                    else:
                        sc = 3 * m + nact
                        nact += 1
                        nc.scalar.activation(
                            E[:, col:col + CHUNK], ps[:],
                            mybir.ActivationFunctionType.Exp,
                            scale=float(1.0 / TAU), bias=biast[:],
                            accum_out=acc[:, sc:sc + 1],
                        )
                # row max over the full row (accum follows op1 = max)
                nc.vector.tensor_scalar(
                    out=E[:], in0=E[:],
                    scalar1=1.0, scalar2=0.0,
                    op0=Alu.mult, op1=Alu.max,
                    accum_out=acc[:, 4 * NBLK + m:4 * NBLK + m + 1],
                )

            nc.sync.dma_start(out_d[:], acc[:])
    nc.compile()
    return nc


def _prep_inputs(features: np.ndarray, alpha):
    feats = np.ascontiguousarray(np.asarray(features, dtype=np.float32))
    assert feats.shape == (N, D), feats.shape
    a = float(np.asarray(alpha, dtype=np.float32))

    norms = np.sqrt((feats.astype(np.float64) ** 2).sum(axis=1, keepdims=True))
    norms = np.maximum(norms, 1e-12)
    fn64 = feats / norms
    fT = np.ascontiguousarray(fn64.T.astype(np.float16))  # [128, 8192] fp16

    ident = np.eye(128, dtype=np.float16)
    negd = (np.eye(128) * DIAG_NEG).astype(np.float16)

    in_maps = []
    for ci in range(NCORES):
        ftc = np.ascontiguousarray(np.roll(fT, -ci * R, axis=1))
        in_maps.append({"ft": ftc, "ident": ident, "negd": negd})
    return in_maps, a, fn64


def _assemble(results, alpha: float, fn64: np.ndarray) -> np.float32:
    c = _shift_center(alpha)
    factor = np.exp(np.float64(c) / TAU)
    thr = np.exp((np.float64(alpha) - c) / TAU)

    S = np.empty(N, np.float64)
    M = np.empty(N, np.float64)
    for ci in range(NCORES):
        o = np.asarray(results[ci]["outPS"], dtype=np.float64)
        Sm = o[:, :3 * NBLK].reshape(128, NBLK, 3).sum(axis=2)
        Sm += o[:, 3 * NBLK:4 * NBLK]
        Mm = o[:, 4 * NBLK:5 * NBLK]
        S[ci * R:(ci + 1) * R] = Sm.T.reshape(R)
        M[ci * R:(ci + 1) * R] = Mm.T.reshape(R)
    S *= factor

    num = np.full(N, EPS)
    den = S + 2.0 * EPS

    # rows that may contain a positive pair (sim >= alpha, off-diagonal):
    # recompute exactly in fp64. Margin covers Schraudolph (-3%) + fp16.
    cand = np.flatnonzero(M >= 0.85 * thr)
    if cand.size:
        sims = fn64[cand] @ fn64.T                       # [ncand, N]
        e = np.exp(sims / TAU)
        e[np.arange(cand.size), cand] = 0.0
        pos = sims >= alpha
        pos[np.arange(cand.size), cand] = False
        P = (e * pos).sum(axis=1)
        Srow = e.sum(axis=1)
        num[cand] = P + EPS
        den[cand] = P + EPS + (Srow - P) + EPS
    loss = np.mean(np.log(den) - np.log(num))
    return np.float32(loss)


def kernel(features, alpha):
    from concourse.bass_utils import run_bass_kernel_spmd

    global LAST_RESULT
    in_maps, a, fn64 = _prep_inputs(features, alpha)
    if a not in _CACHE:
        _CACHE[a] = _build(a)
    nc = _CACHE[a]
    res = run_bass_kernel_spmd(
        nc, in_maps, list(range(NCORES)), trace=PROFILE
    )
    LAST_RESULT = res
    return _assemble(res.results, a, fn64)


# revision 3
# speedup vs baseline: 1.5400x; 1.5400x over previous
"""Trainium2 Bass kernel for nn_CustomLoss_50843822850472.

Computes, for L2-normalized rows f of `features` [8192, 128]:
    sim = f @ f.T                       (diagonal excluded)
    E   = exp((sim - c)/TAU)            (c = shift center, host rescales)
    S_i = sum_j E_ij                    (total mass)
    loss = mean_i [ log(den_i) - log(num_i) ]

Rows are split across 8 NeuronCores (1024 rows/core, 8 blocks of 128).
Each core gets the full feature matrix pre-transposed to [D=128, N=8192]
fp16 and column-rotated by its row offset (SPMD: the diagonal block of
row-block m lands at local columns [m*128, m*128+128), inside chunk 0,
where an accumulate-matmul adds -60000*I so exp -> 0 and the diagonal
drops out of every reduction).

Per row-block m (PSUM chunks of 2048 cols, double-buffered):
  - chunks 0,1,3 and the first half of chunk 2 -> ACT:
    E = exp((sim - c)/TAU) -> fp16 (no accum: reductions are done by a
    DVE pairwise tree, so ACT runs at pure streaming rate)
  - second half of chunk 2 (1024 cols) -> DVE Schraudolph exp:
    bits = u16(sim*slope + B0); the f32->u16 convert saturates (negatives
    clamp to 0 = +0.0 in fp16) and rounds to nearest, so the fp16 view of
    the bits is exp((sim-c)/TAU) within ~3%.  This offloads ACT.
  - a 4-level pairwise fp16 add-tree on DVE (tensor_tensor, 2x mode)
    folds the row [128, 8192] -> [128, 512]; column g of the result is
    sum over {E[g + 512k]}.  The [128, 512] block is DMA'd out per block.

Host: S_row = sum of the 512 partials (fp64) * exp(c/TAU).  Detection:
any partial >= 0.8*exp((alpha-c)/TAU) means the row may contain a
positive pair (a positive contributes >= ~1x threshold to its group,
group background is ~0.03 for alpha=0.5); flagged rows (~180 of 8192)
are recomputed exactly in fp64 on the host, all other rows have P = 0
exactly (num = EPS), matching the reference.
"""
import sys

sys.path.insert(0, "/opt/trn_rl_repo")

import numpy as np

TAU = 0.07
EPS = 1e-10
DIAG_NEG = -60000.0

N = 8192
D = 128
NCORES = 8
R = N // NCORES          # rows per core
NBLK = R // 128          # row blocks per core
CHUNK = 2048             # columns per PSUM chunk (4 banks)
NCHUNK = N // CHUNK
DVE_COLS = 1024          # columns of chunk 2 converted on DVE
TREE_OUT = 512           # tree output width per block
LOG2E = float(np.log2(np.e))
_CACHE = {}
LAST_RESULT = None
PROFILE = False


def _shift_center(alpha: float) -> float:
    # E = exp((sim - c)/TAU) must fit fp16 (and the Schraudolph bits must
    # stay below fp16-inf = 31744): sim <= ~1.0002 needs c >= ~0.23.
    return float(min(max(alpha, 0.30), 1.0))


def _build(alpha: float):
    import concourse.mybir as mybir
    from concourse import bacc, tile

    f32 = mybir.dt.float32
    f16 = mybir.dt.float16
    u16 = mybir.dt.uint16
    Alu = mybir.AluOpType

    c = _shift_center(alpha)
    bias = float(-c / TAU)
    slope = float(1024.0 * LOG2E / TAU)
    b0 = float(15360.0 - c * slope - 44.5)

    nc = bacc.Bacc(
        "TRN2", target_bir_lowering=False, debug=False, num_devices=NCORES
    )
    ft_d = nc.dram_tensor("ft", [128, N], f16, kind="ExternalInput")
    ident_d = nc.dram_tensor("ident", [128, 128], f16, kind="ExternalInput")
    negd_d = nc.dram_tensor("negd", [128, 128], f16, kind="ExternalInput")
    out_d = nc.dram_tensor(
        "treeS", [128, NBLK * TREE_OUT], f16, kind="ExternalOutput"
    )

    with tile.TileContext(nc) as tc:
        with (
            tc.tile_pool(name="sb", bufs=1) as sb,
            tc.tile_pool(name="ep", bufs=2) as ep,
            tc.tile_pool(name="tp", bufs=2) as tp,
            tc.tile_pool(name="pp", bufs=2, space="PSUM") as pp,
        ):
            ft = sb.tile([128, N], f16)
            pieces = [(0, 512), (512, 2048), (2048, 4096), (4096, 6144),
                      (6144, 8192)]
            nc.sync.dma_start(ft[:, 0:512], ft_d[:, 0:512])
            ident = sb.tile([128, 128], f16)
            nc.scalar.dma_start(ident[:], ident_d[:])
            negd = sb.tile([128, 128], f16)
            nc.scalar.dma_start(negd[:], negd_d[:])
            for lo, hi in pieces[1:]:
                nc.sync.dma_start(ft[:, lo:hi], ft_d[:, lo:hi])

            biast = sb.tile([128, 1], f32)
            nc.vector.memset(biast[:], bias)

            asplit = CHUNK - DVE_COLS     # ACT's share of chunk 2

            for m in range(NBLK):
                E = ep.tile([128, N], f16)
                d0 = m * 128
                qd = d0 // 512
                for k in range(NCHUNK):
                    ps = pp.tile([128, CHUNK], f32, tag="ps")
                    for q in range(CHUNK // 512):
                        nc.tensor.matmul(
                            ps[:, q * 512:(q + 1) * 512],
                            lhsT=ft[:, m * 128:(m + 1) * 128],
                            rhs=ft[:, k * CHUNK + q * 512:k * CHUNK + (q + 1) * 512],
                            start=True,
                            stop=not (k == 0 and q == qd),
                        )
                        if k == 0 and q == qd:
                            # accumulate -60000 onto the diagonal 128 cols
                            nc.tensor.matmul(
                                ps[:, d0:d0 + 128],
                                lhsT=ident[:], rhs=negd[:],
                                start=False, stop=True,
                            )
                    col = k * CHUNK
                    if k == 2:
                        nc.scalar.activation(
                            E[:, col:col + asplit], ps[:, 0:asplit],
                            mybir.ActivationFunctionType.Exp,
                            scale=float(1.0 / TAU), bias=biast[:],
                        )
                        # Schraudolph exp: u16(sim*slope + b0), saturating RNE
                        nc.vector.tensor_scalar(
                            out=E[:, col + asplit:col + CHUNK].bitcast(u16),
                            in0=ps[:, asplit:CHUNK],
                            scalar1=slope, scalar2=b0,
                            op0=Alu.mult, op1=Alu.add,
                        )
                    else:
                        nc.scalar.activation(
                            E[:, col:col + CHUNK], ps[:],
                            mybir.ActivationFunctionType.Exp,
                            scale=float(1.0 / TAU), bias=biast[:],
                        )
                # pairwise fp16 sum tree on DVE: 8192 -> 512 (2x mode TT)
                t1 = tp.tile([128, 4096], f16, tag="t1")
                nc.vector.tensor_tensor(
                    out=t1[:], in0=E[:, 0:4096], in1=E[:, 4096:8192],
                    op=Alu.add)
                t2 = tp.tile([128, 2048], f16, tag="t2")
                nc.vector.tensor_tensor(
                    out=t2[:], in0=t1[:, 0:2048], in1=t1[:, 2048:4096],
                    op=Alu.add)
                t3 = tp.tile([128, 1024], f16, tag="t3")
                nc.vector.tensor_tensor(
                    out=t3[:], in0=t2[:, 0:1024], in1=t2[:, 1024:2048],
                    op=Alu.add)
                t4 = tp.tile([128, TREE_OUT], f16, tag="t4")
                nc.vector.tensor_tensor(
                    out=t4[:], in0=t3[:, 0:512], in1=t3[:, 512:1024],
                    op=Alu.add)
                nc.sync.dma_start(
                    out_d[:, m * TREE_OUT:(m + 1) * TREE_OUT], t4[:])
    nc.compile()
    return nc


def _prep_inputs(features: np.ndarray, alpha):
    feats = np.ascontiguousarray(np.asarray(features, dtype=np.float32))
    assert feats.shape == (N, D), feats.shape
    a = float(np.asarray(alpha, dtype=np.float32))

    norms = np.sqrt((feats.astype(np.float64) ** 2).sum(axis=1, keepdims=True))
    norms = np.maximum(norms, 1e-12)
    fn64 = feats / norms
    fT = np.ascontiguousarray(fn64.T.astype(np.float16))  # [128, 8192] fp16

    ident = np.eye(128, dtype=np.float16)
    negd = (np.eye(128) * DIAG_NEG).astype(np.float16)

    in_maps = []
    for ci in range(NCORES):
        ftc = np.ascontiguousarray(np.roll(fT, -ci * R, axis=1))
        in_maps.append({"ft": ftc, "ident": ident, "negd": negd})
    return in_maps, a, fn64


def _assemble(results, alpha: float, fn64: np.ndarray) -> np.float32:
    c = _shift_center(alpha)
    factor = np.exp(np.float64(c) / TAU)
    thr = 0.80 * np.exp((np.float64(alpha) - c) / TAU)

    S = np.empty(N, np.float64)
    cand = np.empty(N, bool)
    for ci in range(NCORES):
        tS = np.asarray(results[ci]["treeS"]).reshape(128, NBLK, TREE_OUT)
        S[ci * R:(ci + 1) * R] = (
            tS.astype(np.float64).sum(axis=2).T.reshape(R) * factor)
        cand[ci * R:(ci + 1) * R] = (
            (tS.astype(np.float32) >= thr).any(axis=2).T.reshape(R))

    num = np.full(N, EPS)
    den = S + 2.0 * EPS

    idx = np.flatnonzero(cand)
    if idx.size:
        sims = fn64[idx] @ fn64.T                        # [ncand, N] fp64
        e = np.exp(sims / TAU)
        e[np.arange(idx.size), idx] = 0.0
        pos = sims >= alpha
        pos[np.arange(idx.size), idx] = False
        P = (e * pos).sum(axis=1)
        Srow = e.sum(axis=1)
        num[idx] = P + EPS
        den[idx] = P + EPS + (Srow - P) + EPS
    loss = np.mean(np.log(den) - np.log(num))
    return np.float32(loss)


def kernel(features, alpha):
    from concourse.bass_utils import run_bass_kernel_spmd

    global LAST_RESULT
    in_maps, a, fn64 = _prep_inputs(features, alpha)
    if a not in _CACHE:
        _CACHE[a] = _build(a)
    nc = _CACHE[a]
    res = run_bass_kernel_spmd(
        nc, in_maps, list(range(NCORES)), trace=PROFILE
    )
    LAST_RESULT = res
    return _assemble(res.results, a, fn64)


# revision 5
# speedup vs baseline: 1.6473x; 1.0696x over previous
"""Trainium2 Bass kernel for nn_CustomLoss_50843822850472.

Computes, for L2-normalized rows f of `features` [8192, 128]:
    sim = f @ f.T                       (diagonal excluded)
    E   = exp((sim - c)/TAU)            (c = shift center, host rescales)
    S_i = sum_j E_ij                    (total mass)
    loss = mean_i [ log(den_i) - log(num_i) ]

Rows are split across 8 NeuronCores (1024 rows/core, 8 blocks of 128).
Each core gets the full feature matrix pre-transposed to [D=128, N=8192]
fp16 and column-rotated by its row offset (SPMD: the diagonal block of
row-block m lands at local columns [m*128, m*128+128), inside chunk 0,
where an accumulate-matmul adds -60000*I so exp -> 0 and the diagonal
drops out of every reduction).

Per row-block m (PSUM chunks of 2048 cols, double-buffered):
  - chunks 0,1,3 and the first half of chunk 2 -> ACT:
    E = exp((sim - c)/TAU) -> fp16 (no accum: reductions are done by a
    DVE pairwise tree, so ACT runs at pure streaming rate)
  - second half of chunk 2 (1024 cols) -> DVE Schraudolph exp:
    bits = u16(sim*slope + B0); the f32->u16 convert saturates (negatives
    clamp to 0 = +0.0 in fp16) and rounds to nearest, so the fp16 view of
    the bits is exp((sim-c)/TAU) within ~3%.  This offloads ACT.
  - a 4-level pairwise fp16 add-tree on DVE (tensor_tensor, 2x mode)
    folds the row [128, 8192] -> [128, 512]; column g of the result is
    sum over {E[g + 512k]}.  The [128, 512] block is DMA'd out per block.

Host: S_row = sum of the 512 partials (fp64) * exp(c/TAU).  Detection:
any partial >= 0.8*exp((alpha-c)/TAU) means the row may contain a
positive pair (a positive contributes >= ~1x threshold to its group,
group background is ~0.03 for alpha=0.5); flagged rows (~180 of 8192)
are recomputed exactly in fp64 on the host, all other rows have P = 0
exactly (num = EPS), matching the reference.
"""
import sys

sys.path.insert(0, "/opt/trn_rl_repo")

import numpy as np

TAU = 0.07
EPS = 1e-10
DIAG_NEG = -60000.0

N = 8192
D = 128
NCORES = 8
R = N // NCORES          # rows per core
NBLK = R // 128          # row blocks per core
CHUNK = 2048             # columns per PSUM chunk (4 banks)
NCHUNK = N // CHUNK
DVE_COLS = 2048          # columns of chunk 2 converted on DVE
TREE_OUT = 2048          # tree output width per block
LOG2E = float(np.log2(np.e))
_CACHE = {}
LAST_RESULT = None
PROFILE = False


def _shift_center(alpha: float) -> float:
    # E = exp((sim - c)/TAU) must fit fp16 (and the Schraudolph bits must
    # stay below fp16-inf = 31744): sim <= ~1.0002 needs c >= ~0.23.
    return float(min(max(alpha, 0.30), 1.0))


def _build(alpha: float):
    import concourse.mybir as mybir
    from concourse import bacc, tile

    f32 = mybir.dt.float32
    f16 = mybir.dt.float16
    u16 = mybir.dt.uint16
    Alu = mybir.AluOpType

    c = _shift_center(alpha)
    bias = float(-c / TAU)
    slope = float(1024.0 * LOG2E / TAU)
    b0 = float(15360.0 - c * slope - 44.5)

    nc = bacc.Bacc(
        "TRN2", target_bir_lowering=False, debug=False, num_devices=NCORES
    )
    ft_d = nc.dram_tensor("ft", [128, N], f16, kind="ExternalInput")
    ident_d = nc.dram_tensor("ident", [128, 128], f16, kind="ExternalInput")
    negd_d = nc.dram_tensor("negd", [128, 128], f16, kind="ExternalInput")
    out_d = nc.dram_tensor(
        "treeS", [128, NBLK * TREE_OUT], f16, kind="ExternalOutput"
    )

    with tile.TileContext(nc) as tc:
        with (
            tc.tile_pool(name="sb", bufs=1) as sb,
            tc.tile_pool(name="ep", bufs=2) as ep,
            tc.tile_pool(name="tp", bufs=2) as tp,
            tc.tile_pool(name="pp", bufs=2, space="PSUM") as pp,
        ):
            ft = sb.tile([128, N], f16)
            pieces = [(0, 512), (512, 2048), (2048, 4096), (4096, 6144),
                      (6144, 8192)]
            nc.sync.dma_start(ft[:, 0:512], ft_d[:, 0:512])
            ident = sb.tile([128, 128], f16)
            nc.scalar.dma_start(ident[:], ident_d[:])
            negd = sb.tile([128, 128], f16)
            nc.scalar.dma_start(negd[:], negd_d[:])
            for lo, hi in pieces[1:]:
                nc.sync.dma_start(ft[:, lo:hi], ft_d[:, lo:hi])

            biast = sb.tile([128, 1], f32)
            nc.vector.memset(biast[:], bias)

            # software-pipelined tree state from the previous block
            prev = None     # (E, t1a, m)

            def finish_tree(E, t1a, m):
                # T1b: right half pair (chunk2-cvt cols + chunk3 cols)
                t1b = tp.tile([128, 2048], f16, tag="t1b")
                nc.vector.tensor_tensor(
                    out=t1b[:], in0=E[:, 4096:6144], in1=E[:, 6144:8192],
                    op=Alu.add)
                t2 = tp.tile([128, TREE_OUT], f16, tag="t2")
                nc.vector.tensor_tensor(
                    out=t2[:], in0=t1a[:], in1=t1b[:], op=Alu.add)
                nc.sync.dma_start(
                    out_d[:, m * TREE_OUT:(m + 1) * TREE_OUT], t2[:])

            for m in range(NBLK):
                if prev is not None:
                    finish_tree(*prev)
                E = ep.tile([128, N], f16)
                t1a = None
                d0 = m * 128
                qd = d0 // 512
                for k in range(NCHUNK):
                    ps = pp.tile([128, CHUNK], f32, tag="ps")
                    for q in range(CHUNK // 512):
                        nc.tensor.matmul(
                            ps[:, q * 512:(q + 1) * 512],
                            lhsT=ft[:, m * 128:(m + 1) * 128],
                            rhs=ft[:, k * CHUNK + q * 512:k * CHUNK + (q + 1) * 512],
                            start=True,
                            stop=not (k == 0 and q == qd),
                        )
                        if k == 0 and q == qd:
                            # accumulate -60000 onto the diagonal 128 cols
                            nc.tensor.matmul(
                                ps[:, d0:d0 + 128],
                                lhsT=ident[:], rhs=negd[:],
                                start=False, stop=True,
                            )
                    col = k * CHUNK
                    if k == 2:
                        # Schraudolph exp: u16(sim*slope + b0), saturating RNE
                        nc.vector.tensor_scalar(
                            out=E[:, col:col + CHUNK].bitcast(u16),
                            in0=ps[:],
                            scalar1=slope, scalar2=b0,
                            op0=Alu.mult, op1=Alu.add,
                        )
                        # T1a: left half pair (ACT chunks 0 and 1), runs as
                        # soon as those are written; overlaps ACT chunk 3
                        t1a = tp.tile([128, 2048], f16, tag="t1a")
                        nc.vector.tensor_tensor(
                            out=t1a[:], in0=E[:, 0:2048], in1=E[:, 2048:4096],
                            op=Alu.add)
                    else:
                        nc.scalar.activation(
                            E[:, col:col + CHUNK], ps[:],
                            mybir.ActivationFunctionType.Exp,
                            scale=float(1.0 / TAU), bias=biast[:],
                        )
                prev = (E, t1a, m)
            finish_tree(*prev)
    nc.compile()
    return nc


def _prep_inputs(features: np.ndarray, alpha):
    feats = np.ascontiguousarray(np.asarray(features, dtype=np.float32))
    assert feats.shape == (N, D), feats.shape
    a = float(np.asarray(alpha, dtype=np.float32))

    norms = np.sqrt((feats.astype(np.float64) ** 2).sum(axis=1, keepdims=True))
    norms = np.maximum(norms, 1e-12)
    fn64 = feats / norms
    fT = np.ascontiguousarray(fn64.T.astype(np.float16))  # [128, 8192] fp16

    ident = np.eye(128, dtype=np.float16)
    negd = (np.eye(128) * DIAG_NEG).astype(np.float16)

    in_maps = []
    for ci in range(NCORES):
        ftc = np.ascontiguousarray(np.roll(fT, -ci * R, axis=1))
        in_maps.append({"ft": ftc, "ident": ident, "negd": negd})
    return in_maps, a, fn64


def _assemble(results, alpha: float, fn64: np.ndarray) -> np.float32:
    c = _shift_center(alpha)
    factor = np.exp(np.float64(c) / TAU)
    thr = 0.80 * np.exp((np.float64(alpha) - c) / TAU)

    S = np.empty(N, np.float64)
    cand = np.empty(N, bool)
    for ci in range(NCORES):
        tS = np.asarray(results[ci]["treeS"]).reshape(128, NBLK, TREE_OUT)
        S[ci * R:(ci + 1) * R] = (
            tS.astype(np.float64).sum(axis=2).T.reshape(R) * factor)
        cand[ci * R:(ci + 1) * R] = (
            (tS.astype(np.float32) >= thr).any(axis=2).T.reshape(R))

    num = np.full(N, EPS)
    den = S + 2.0 * EPS

    idx = np.flatnonzero(cand)
    if idx.size:
        sims = fn64[idx] @ fn64.T                        # [ncand, N] fp64
        e = np.exp(sims / TAU)
        e[np.arange(idx.size), idx] = 0.0
        pos = sims >= alpha
        pos[np.arange(idx.size), idx] = False
        P = (e * pos).sum(axis=1)
        Srow = e.sum(axis=1)
        num[idx] = P + EPS
        den[idx] = P + EPS + (Srow - P) + EPS
    loss = np.mean(np.log(den) - np.log(num))
    return np.float32(loss)


def kernel(features, alpha):
    from concourse.bass_utils import run_bass_kernel_spmd

    global LAST_RESULT
    in_maps, a, fn64 = _prep_inputs(features, alpha)
    if a not in _CACHE:
        _CACHE[a] = _build(a)
    nc = _CACHE[a]
    res = run_bass_kernel_spmd(
        nc, in_maps, list(range(NCORES)), trace=PROFILE
    )
    LAST_RESULT = res
    return _assemble(res.results, a, fn64)


# revision 7
# speedup vs baseline: 1.8238x; 1.1072x over previous
"""Trainium2 Bass kernel for nn_CustomLoss_50843822850472.

Computes, for L2-normalized rows f of `features` [8192, 128]:
    sim = f @ f.T                       (diagonal excluded)
    E   = exp((sim - c)/TAU)            (c = shift center, host rescales)
    S_i = sum_j E_ij                    (total mass)
    loss = mean_i [ log(den_i) - log(num_i) ]

Rows are split across 8 NeuronCores (1024 rows/core, 8 blocks of 128).
Each core gets the full feature matrix pre-transposed to [D=128, N=8192]
fp16 and column-rotated by its row offset (SPMD: the diagonal block of
row-block m lands at local columns [m*128, m*128+128), inside chunk 0,
where an accumulate-matmul adds -60000*I so exp -> 0 and the diagonal
drops out of every reduction).

Per row-block m (PSUM chunks of 2048 cols, double-buffered):
  - chunks 0,1,3 and the first half of chunk 2 -> ACT:
    E = exp((sim - c)/TAU) -> fp16 (no accum: reductions are done by a
    DVE pairwise tree, so ACT runs at pure streaming rate)
  - second half of chunk 2 (1024 cols) -> DVE Schraudolph exp:
    bits = u16(sim*slope + B0); the f32->u16 convert saturates (negatives
    clamp to 0 = +0.0 in fp16) and rounds to nearest, so the fp16 view of
    the bits is exp((sim-c)/TAU) within ~3%.  This offloads ACT.
  - a 4-level pairwise fp16 add-tree on DVE (tensor_tensor, 2x mode)
    folds the row [128, 8192] -> [128, 512]; column g of the result is
    sum over {E[g + 512k]}.  The [128, 512] block is DMA'd out per block.

Host: S_row = sum of the 512 partials (fp64) * exp(c/TAU).  Detection:
any partial >= 0.8*exp((alpha-c)/TAU) means the row may contain a
positive pair (a positive contributes >= ~1x threshold to its group,
group background is ~0.03 for alpha=0.5); flagged rows (~180 of 8192)
are recomputed exactly in fp64 on the host, all other rows have P = 0
exactly (num = EPS), matching the reference.
"""
import sys

sys.path.insert(0, "/opt/trn_rl_repo")

import numpy as np

TAU = 0.07
EPS = 1e-10
DIAG_NEG = -60000.0

N = 8192
D = 128
NCORES = 8
R = N // NCORES          # rows per core
NBLK = R // 128          # row blocks per core
CHUNK = 2048             # columns per PSUM chunk (4 banks)
NCHUNK = N // CHUNK
DVE_COLS = 2048          # columns of chunk 2 converted on DVE
TREE_OUT = 2048          # tree output width per block
LOG2E = float(np.log2(np.e))
_CACHE = {}
LAST_RESULT = None
PROFILE = False


def _shift_center(alpha: float) -> float:
    # E = exp((sim - c)/TAU) must fit fp16 (and the Schraudolph bits must
    # stay below fp16-inf = 31744): sim <= ~1.0002 needs c >= ~0.23.
    return float(min(max(alpha, 0.30), 1.0))


def _build(alpha: float):
    import concourse.mybir as mybir
    from concourse import bacc, tile

    f32 = mybir.dt.float32
    f16 = mybir.dt.float16
    u16 = mybir.dt.uint16
    Alu = mybir.AluOpType

    c = _shift_center(alpha)
    bias = float(-c / TAU)
    slope = float(1024.0 * LOG2E / TAU)
    b0 = float(15360.0 - c * slope - 44.5)

    nc = bacc.Bacc(
        "TRN2", target_bir_lowering=False, debug=False, num_devices=NCORES
    )
    ft_d = nc.dram_tensor("ft", [128, N], f16, kind="ExternalInput")
    ident_d = nc.dram_tensor("ident", [128, 128], f16, kind="ExternalInput")
    negd_d = nc.dram_tensor("negd", [128, 128], f16, kind="ExternalInput")
    out_d = nc.dram_tensor(
        "treeS", [128, NBLK * TREE_OUT], f16, kind="ExternalOutput"
    )

    with tile.TileContext(nc) as tc:
        with (
            tc.tile_pool(name="sb", bufs=1) as sb,
            tc.tile_pool(name="ep", bufs=2) as ep,
            tc.tile_pool(name="tp", bufs=2) as tp,
            tc.tile_pool(name="ppa", bufs=2, space="PSUM") as ppa,
            tc.tile_pool(name="ppd", bufs=1, space="PSUM") as ppd,
        ):
            ft = sb.tile([128, N], f16)
            pieces = [(0, 512), (512, 2048), (2048, 4096), (4096, 6144),
                      (6144, 8192)]
            nc.sync.dma_start(ft[:, 0:512], ft_d[:, 0:512])
            ident = sb.tile([128, 128], f16)
            nc.scalar.dma_start(ident[:], ident_d[:])
            negd = sb.tile([128, 128], f16)
            nc.scalar.dma_start(negd[:], negd_d[:])
            for lo, hi in pieces[1:]:
                nc.sync.dma_start(ft[:, lo:hi], ft_d[:, lo:hi])

            biast = sb.tile([128, 1], f32)
            nc.vector.memset(biast[:], bias)

            # software-pipelined tree state from the previous block
            prev = None     # (E, t1a, m)

            def finish_tree(E, t1a, m):
                # T1b: right half pair (chunk2-cvt cols + chunk3 cols)
                t1b = tp.tile([128, 2048], f16, tag="t1b")
                nc.vector.tensor_tensor(
                    out=t1b[:], in0=E[:, 4096:6144], in1=E[:, 6144:8192],
                    op=Alu.add)
                t2 = tp.tile([128, TREE_OUT], f16, tag="t2")
                nc.vector.tensor_tensor(
                    out=t2[:], in0=t1a[:], in1=t1b[:], op=Alu.add)
                nc.sync.dma_start(
                    out_d[:, m * TREE_OUT:(m + 1) * TREE_OUT], t2[:])

            # per-block column layout: 4 ACT chunks of 1536 ([0:6144]) and
            # 2 DVE cvt chunks of 1024 ([6144:8192]).  PSUM: 4*3 + 2 = 8
            # banks, so the block m+1 chunk-0 buffer is freed by an EARLY
            # ACT chunk of block m, not by the last consumer (no boundary
            # stall).
            ACHUNK = 1536
            DCHUNK = 1024

            def mains(ps, m, lo, cols):
                d0 = m * 128
                for q in range(cols // 512):
                    nc.tensor.matmul(
                        ps[:, q * 512:(q + 1) * 512],
                        lhsT=ft[:, m * 128:(m + 1) * 128],
                        rhs=ft[:, lo + q * 512:lo + (q + 1) * 512],
                        start=True,
                        stop=not (lo + q * 512 <= d0 < lo + (q + 1) * 512),
                    )
                    if lo + q * 512 <= d0 < lo + (q + 1) * 512:
                        # accumulate -60000 onto the diagonal 128 cols
                        nc.tensor.matmul(
                            ps[:, d0 - lo:d0 - lo + 128],
                            lhsT=ident[:], rhs=negd[:],
                            start=False, stop=True,
                        )

            for m in range(NBLK):
                E = ep.tile([128, N], f16)
                # ACT chunks 0,1 + first DVE chunk fill
                psa = []
                for k in range(2):
                    ps = ppa.tile([128, ACHUNK], f32, tag="psa")
                    mains(ps, m, k * ACHUNK, ACHUNK)
                    psa.append(ps)
                pd1 = ppd.tile([128, DCHUNK], f32, tag="psd")
                mains(pd1, m, 6144, DCHUNK)
                for k in range(2):
                    nc.scalar.activation(
                        E[:, k * ACHUNK:(k + 1) * ACHUNK], psa[k][:],
                        mybir.ActivationFunctionType.Exp,
                        scale=float(1.0 / TAU), bias=biast[:],
                    )
                # ACT chunks 2,3 + second DVE chunk fill
                psb = []
                for k in range(2, 4):
                    ps = ppa.tile([128, ACHUNK], f32, tag="psa")
                    mains(ps, m, k * ACHUNK, ACHUNK)
                    psb.append(ps)
                pd2 = ppd.tile([128, DCHUNK], f32, tag="psd")
                mains(pd2, m, 6144 + DCHUNK, DCHUNK)
                if prev is not None:
                    finish_tree(*prev)
                # Schraudolph exp: u16(sim*slope + b0), saturating RNE
                nc.vector.tensor_scalar(
                    out=E[:, 6144:6144 + DCHUNK].bitcast(u16),
                    in0=pd1[:],
                    scalar1=slope, scalar2=b0,
                    op0=Alu.mult, op1=Alu.add,
                )
                for k in range(2, 4):
                    nc.scalar.activation(
                        E[:, k * ACHUNK:(k + 1) * ACHUNK], psb[k - 2][:],
                        mybir.ActivationFunctionType.Exp,
                        scale=float(1.0 / TAU), bias=biast[:],
                    )
                nc.vector.tensor_scalar(
                    out=E[:, 6144 + DCHUNK:8192].bitcast(u16),
                    in0=pd2[:],
                    scalar1=slope, scalar2=b0,
                    op0=Alu.mult, op1=Alu.add,
                )
                # T1a: left half pair, ready once ACT chunks 0-2 are written
                t1a = tp.tile([128, 2048], f16, tag="t1a")
                nc.vector.tensor_tensor(
                    out=t1a[:], in0=E[:, 0:2048], in1=E[:, 2048:4096],
                    op=Alu.add)
                prev = (E, t1a, m)
            finish_tree(*prev)
    nc.compile()
    return nc


def _prep_inputs(features: np.ndarray, alpha):
    feats = np.ascontiguousarray(np.asarray(features, dtype=np.float32))
    assert feats.shape == (N, D), feats.shape
    a = float(np.asarray(alpha, dtype=np.float32))

    norms = np.sqrt((feats.astype(np.float64) ** 2).sum(axis=1, keepdims=True))
    norms = np.maximum(norms, 1e-12)
    fn64 = feats / norms
    fT = np.ascontiguousarray(fn64.T.astype(np.float16))  # [128, 8192] fp16

    ident = np.eye(128, dtype=np.float16)
    negd = (np.eye(128) * DIAG_NEG).astype(np.float16)

    in_maps = []
    for ci in range(NCORES):
        ftc = np.ascontiguousarray(np.roll(fT, -ci * R, axis=1))
        in_maps.append({"ft": ftc, "ident": ident, "negd": negd})
    return in_maps, a, fn64


def _assemble(results, alpha: float, fn64: np.ndarray) -> np.float32:
    c = _shift_center(alpha)
    factor = np.exp(np.float64(c) / TAU)
    thr = 0.80 * np.exp((np.float64(alpha) - c) / TAU)

    S = np.empty(N, np.float64)
    cand = np.empty(N, bool)
    for ci in range(NCORES):
        tS = np.asarray(results[ci]["treeS"]).reshape(128, NBLK, TREE_OUT)
        S[ci * R:(ci + 1) * R] = (
            tS.astype(np.float64).sum(axis=2).T.reshape(R) * factor)
        cand[ci * R:(ci + 1) * R] = (
            (tS.astype(np.float32) >= thr).any(axis=2).T.reshape(R))

    num = np.full(N, EPS)
    den = S + 2.0 * EPS

    idx = np.flatnonzero(cand)
    if idx.size:
        sims = fn64[idx] @ fn64.T                        # [ncand, N] fp64
        e = np.exp(sims / TAU)
        e[np.arange(idx.size), idx] = 0.0
        pos = sims >= alpha
        pos[np.arange(idx.size), idx] = False
        P = (e * pos).sum(axis=1)
        Srow = e.sum(axis=1)
        num[idx] = P + EPS
        den[idx] = P + EPS + (Srow - P) + EPS
    loss = np.mean(np.log(den) - np.log(num))
    return np.float32(loss)


def kernel(features, alpha):
    from concourse.bass_utils import run_bass_kernel_spmd

    global LAST_RESULT
    in_maps, a, fn64 = _prep_inputs(features, alpha)
    if a not in _CACHE:
        _CACHE[a] = _build(a)
    nc = _CACHE[a]
    res = run_bass_kernel_spmd(
        nc, in_maps, list(range(NCORES)), trace=PROFILE
    )
    LAST_RESULT = res
    return _assemble(res.results, a, fn64)
